# revision 48
# baseline (speedup 1.0000x reference)
"""Multi-head causal attention (B=2, T=2048, C=2048, 16 heads, fp32) on 8
Trainium2 NeuronCores.

Sharding: data-parallel over batch (2) x tensor-parallel over heads
(4 heads/core).  Core c handles batch c//4, heads 4*(c%4)..4*(c%4)+3.
Each core computes q/k/v projections for its heads, causal softmax
attention, and a partial output projection (its heads' rows of Wout);
the host sums the 4 partials per batch.

Design notes:
  * All matmul operands are bf16 (PE runs bf16 at the same 1 col/cycle
    as fp32r, but DMA bytes halve and SBUF capacity doubles).  PSUM
    accumulation stays fp32.
  * q^T, k^T, v are kept fully resident in SBUF, written directly from
    the projection PSUM drains -- no DRAM round-trip.
  * Projection slabs and attention blocks are interleaved
    (A0 A1 D0 A2 D1 A3 D2 D3) so the PE instruction stream never waits
    at a phase boundary.
  * Softmax denominators: per-j exp tiles are accumulated elementwise
    into a bf16 [128,512] partial on the DVE (2x 16-bit rate), then ONE
    ones-matmul per (block,head) does the 128-partition sum in fp32
    PSUM.  This moves ~61k PE cycles (~26us) of per-j ones-matmuls off
    the Tensor engine.  The ones-matmul + reciprocal + rescale for head
    h are emitted two j-steps into head h+1 so the DVE accumulation
    chain never blocks the PE queue head.
  * ACT throughput is the binding constraint of the attention phase
    once the ones-matmuls are gone (PE 432ns vs exp 720ns per tile), so
    every PSUM tile is a 2-bank pair [128,2,512] and non-diagonal score
    tiles are exp'd 1024 wide in a single ACTIVATE (573ns/tile: the
    352-cycle ACTIVATE overhead amortizes over two tiles).  Projection
    drains pair the same way (two heads / two t-tiles per ACTIVATE) and
    run on ACT, which is idle during phase A.
  * av matmuls lag the exp stream by one pair so the PE never waits on
    a freshly-written at tile.
  * Out-projection PSUM pairs drain with ACT on half 0 and DVE on half
    1 concurrently; block b-1's out projection is emitted one il-slice
    per head of block b so this work spreads across the block.
  * Diagonal-crossing score tiles are column-trimmed: for diagonal tile
    g=j-4b>0, columns tq_local<128g are fully masked, so scores / exp /
    mask / attn@v / denominator all operate on [128g:512] only.
  * Startup: DMA fans across sync+scalar (slab/wv, small first chunks)
    and gpsimd (wqk); ~20 warmup matmuls on a constant tile keep the
    HAM activity monitor busy through the DMA wait so the PE clock gate
    is at 2.4 GHz when real data lands.
  * Outputs are written bf16 (host upconverts and reduces in f64); the
    final row's DMA is split fine-grained across two queues to shrink
    the post-compute tail.
"""

import numpy as np

import concourse.bass as bass
import concourse.tile as tile
from concourse import bacc, mybir
from concourse.bass_utils import run_bass_kernel_spmd

B, T, C = 2, 2048, 2048
H, DH = 16, 128
HPC = 4            # heads per core
KO = C // 128      # 16 contraction tiles
NSLAB = 4          # 512-wide t slabs in phase A
SLAB = T // NSLAB  # 512
NB = 4             # 512-wide tq blocks in phase D
BW = T // NB       # 512
NT = T // 128      # 16 t tiles
SCALE = DH ** -0.5
EXP_BIAS = -2.0    # cancels in ps_o/ps_n
F32 = mybir.dt.float32
BF16 = mybir.dt.bfloat16

N_WARMUP = 20      # N=128 warmup matmuls during startup DMA wait


def build_nc():
    nc = bacc.Bacc("TRN2", target_bir_lowering=False, debug=False, num_devices=8)
    # inputs are HOST-PACKED partition-major: per partition row the data a
    # DMA chunk needs is CONTIGUOUS (4-16KB segments instead of 1KB strided),
    # so each dma_start stream runs at far higher bandwidth during startup
    xt_d = nc.dram_tensor("xt", [128, NSLAB * KO * SLAB], BF16, kind="ExternalInput")
    wqk_d = nc.dram_tensor("wqk", [128, KO * 2 * HPC * DH], BF16, kind="ExternalInput")
    wv_d = nc.dram_tensor("wv", [128, KO * HPC * DH], BF16, kind="ExternalInput")
    wout_d = nc.dram_tensor("wout", [128, HPC * C], BF16, kind="ExternalInput")
    out_d = nc.dram_tensor("out", [T, C], BF16, kind="ExternalOutput")

    xt = xt_d.ap().rearrange("p (s ko t) -> p s ko t", s=NSLAB, ko=KO)
    wqk = wqk_d.ap().rearrange("p (ko m) -> p ko m", ko=KO)
    wv = wv_d.ap().rearrange("p (ko m) -> p ko m", ko=KO)
    wout = wout_d.ap().rearrange("p (h c) -> p h c", h=HPC)
    out = out_d.ap()

    with tile.TileContext(nc) as tc:
        from contextlib import ExitStack

        with ExitStack() as top:
            const_pool = top.enter_context(tc.tile_pool(name="const", bufs=1))
            wqk_pool = top.enter_context(tc.tile_pool(name="wqk", bufs=1))
            wv_pool = top.enter_context(tc.tile_pool(name="wv", bufs=1))
            wout_pool = top.enter_context(tc.tile_pool(name="wout", bufs=1))
            qkt_pool = top.enter_context(tc.tile_pool(name="qkt", bufs=2))
            vp_pool = top.enter_context(tc.tile_pool(name="vp", bufs=1))
            slab_pool = top.enter_context(tc.tile_pool(name="slab", bufs=2))
            at_pool = top.enter_context(tc.tile_pool(name="at", bufs=2))
            den_pool = top.enter_context(tc.tile_pool(name="den", bufs=2))
            rec_pool = top.enter_context(tc.tile_pool(name="rec", bufs=2))
            aot_pool = top.enter_context(tc.tile_pool(name="aot", bufs=2))
            oc_pool = top.enter_context(tc.tile_pool(name="oc", bufs=3))
            # every PSUM tile is a 2-bank pair so exps/drains can run 1024
            # wide in one ACTIVATE (amortizes the 352-cycle ACT overhead)
            ps_big = top.enter_context(tc.tile_pool(name="ps_big", bufs=2, space="PSUM"))
            ps_o_pool = top.enter_context(tc.tile_pool(name="ps_o", bufs=2, space="PSUM"))
            # single-bank pool shared by the softmax-denominator matmul
            # target and the out-projection accumulators (keeps ps_big's
            # pair rotation free for the score/exp stream)
            ps_sm_pool = top.enter_context(tc.tile_pool(name="ps_sm", bufs=2, space="PSUM"))

            ones_mat = const_pool.tile([128, 128], BF16)
            nc.vector.memset(ones_mat[:], 1.0)
            bias_sb = const_pool.tile([128, 1], F32)
            nc.vector.memset(bias_sb[:], EXP_BIAS)

            # PE warmup during the startup DMA wait: keeps the HAM activity
            # monitor busy so the clock gate is at 8/8 when real data lands.
            ps_warm = ps_o_pool.tile([128, BW], F32, name="ps_warm", tag="ps_o")
            for _ in range(N_WARMUP):
                nc.tensor.matmul(ps_warm[:, 0:128], ones_mat[:], ones_mat[:],
                                 start=True, stop=True)

            wqk_sb = wqk_pool.tile([128, KO, 2 * HPC * DH], BF16)
            wv_sb = wv_pool.tile([128, KO, HPC * DH], BF16)
            wout_sb = wout_pool.tile([128, HPC, C], BF16)
            # resident q^T / k^T: [d, head, t]
            qt_res = qkt_pool.tile([128, HPC, T], BF16, name="qt_res")
            kt_res = qkt_pool.tile([128, HPC, T], BF16, name="kt_res")
            # resident v: [tk within tile, t-tile, head*d]
            vp_all = vp_pool.tile([128, NT, HPC * DH], BF16, name="vp_all")

            # ================= projections (slab s) =================
            def emit_A_dma(s):
                slab = slab_pool.tile([128, KO, SLAB], BF16)
                if s == 0:
                    # critical first chunks: 1-ko granularity through ko8 on
                    # the two hwdge engines -- a 2-ko chunk is 256KB on one
                    # ~24GB/s stream (~10.5us), which arrives later than the
                    # ko-outer chains consume it
                    for k in range(9):
                        nc.sync.dma_start(slab[:, k:k + 1], xt[:, 0, k:k + 1])
                        nc.scalar.dma_start(wv_sb[:, k:k + 1], wv[:, k:k + 1])
                    engs = [nc.sync, nc.scalar]
                    n = 0
                    for g in range(3):
                        kos = slice(2 * g + 9, 2 * g + 11)
                        engs[n % 2].dma_start(slab[:, kos], xt[:, 0, kos])
                        n += 1
                        engs[n % 2].dma_start(wv_sb[:, kos], wv[:, kos])
                        n += 1
                    engs[n % 2].dma_start(slab[:, 15:16], xt[:, 0, 15:16])
                    engs[(n + 1) % 2].dma_start(wv_sb[:, 15:16], wv[:, 15:16])
                    # wqk (needed only after the v chains): first half on the
                    # gpsimd software DGE, second half at the TAIL of sync's
                    # dispatch queue -- its streams then start ~5us later,
                    # leaving early HBM bandwidth to the critical slab+wv
                    for g in range(4):
                        kos = slice(2 * g, 2 * g + 2)
                        nc.gpsimd.dma_start(wqk_sb[:, kos], wqk[:, kos])
                    for g in range(4, 8):
                        kos = slice(2 * g, 2 * g + 2)
                        nc.sync.dma_start(wqk_sb[:, kos], wqk[:, kos])
                else:
                    # 2-ko first chunks: a 4-ko chunk (512KB, one stream) is
                    # ~23us and arrives after this slab's chains want it
                    for g2 in range(2):
                        kos = slice(2 * g2, 2 * g2 + 2)
                        nc.sync.dma_start(slab[:, kos], xt[:, s, kos])
                    for g4 in range(1, 4):
                        kos = slice(4 * g4, 4 * g4 + 4)
                        nc.sync.dma_start(slab[:, kos], xt[:, s, kos])
                if s == 1:
                    # wout needed only from final_proj(0) during block 1
                    nc.sync.dma_start(wout_sb[:], wout[:])
                return slab

            # drains split ACT half0 / DVE half1 so a pair frees in one
            # drain latency; both engines are mostly idle during phase A
            def drain_pair(pr, dst0, dst1):
                nc.scalar.activation(dst0, pr[:, 0],
                                     mybir.ActivationFunctionType.Copy)
                nc.vector.tensor_copy(dst1, pr[:, 1])

            # DVE-only drain: a chain drain placed on ACT would sit in the
            # ACT FIFO ahead of later exps and block them until the chain's
            # matmuls complete (head-of-line).  On DVE the blocking only
            # delays den accumulation, whose consumer (the ones-matmul) is
            # already deferred a full head.
            def drain_pair_dve(pr, dst0, dst1):
                nc.vector.tensor_copy(dst0, pr[:, 0])
                nc.vector.tensor_copy(dst1, pr[:, 1])

            def emit_A_compute(s, slab):
                # the block before an attention phase ends with drains that
                # must not block that phase's exps on ACT
                dp = drain_pair if s == 0 else drain_pair_dve
                if s == 0:
                    # startup: ko-outer across 4 pair-halves (4 live banks)
                    # so the PE progresses on 4 output tiles per arriving
                    # DMA ko-chunk instead of stalling a serial chain
                    prs = [ps_big.tile([128, 2, HPC * DH], F32, tag="big",
                                       name="ps_v") for _ in range(2)]
                    for ko in range(KO):
                        for tt in range(SLAB // 128):
                            nc.tensor.matmul(
                                prs[tt // 2][:, tt % 2],
                                slab[:, ko, tt * 128:(tt + 1) * 128],
                                wv_sb[:, ko], start=(ko == 0),
                                stop=(ko == KO - 1),
                            )
                    for p in range(2):
                        dp(prs[p], vp_all[:, 2 * p],
                                   vp_all[:, 2 * p + 1])
                else:
                    # steady state: ko-inner pair-chains; the next chain's
                    # matmuls overlap the previous pair's drain (bufs=2)
                    for p in range(2):
                        pr = ps_big.tile([128, 2, HPC * DH], F32, tag="big",
                                         name="ps_v")
                        for ko in range(KO):
                            for half in range(2):
                                tt = 2 * p + half
                                nc.tensor.matmul(
                                    pr[:, half],
                                    slab[:, ko, tt * 128:(tt + 1) * 128],
                                    wv_sb[:, ko], start=(ko == 0),
                                    stop=(ko == KO - 1),
                                )
                        dp(pr, vp_all[:, s * 4 + 2 * p],
                                   vp_all[:, s * 4 + 2 * p + 1])
                for half in range(2):
                    dst = (qt_res if half == 0 else kt_res)
                    for p in range(2):
                        pr = ps_big.tile([128, 2, SLAB], F32, tag="big",
                                         name="ps_qk")
                        for ko in range(KO):
                            for q in range(2):
                                co = HPC * half + 2 * p + q
                                nc.tensor.matmul(
                                    pr[:, q],
                                    wqk_sb[:, ko, co * 128:(co + 1) * 128],
                                    slab[:, ko], start=(ko == 0),
                                    stop=(ko == KO - 1),
                                )
                        dp(
                            pr,
                            dst[:, 2 * p, s * SLAB:(s + 1) * SLAB],
                            dst[:, 2 * p + 1, s * SLAB:(s + 1) * SLAB])

            def make_half_chains(s, slab):
                # slab s's six ko-inner pair-chains, each split into two
                # 16-matmul halves.  Returned as closures to interleave into
                # an attention block as ACT-free PE filler at a granularity
                # fine enough to cover the exp stream's latency within a
                # head iteration (a whole 32-mm chain between heads is too
                # coarse -- the stalls happen inside the head).
                halves = []

                def add_chain(alloc, emit_mms, drain):
                    cell = {}

                    def first():
                        cell["pr"] = alloc()
                        emit_mms(cell["pr"], range(0, KO // 2))

                    def second():
                        emit_mms(cell["pr"], range(KO // 2, KO))
                        drain(cell["pr"])

                    halves.append(first)
                    halves.append(second)

                def v_chain(p):
                    def alloc():
                        return ps_big.tile([128, 2, HPC * DH], F32, tag="big",
                                           name="ps_v")

                    def mms(pr, kos):
                        for ko in kos:
                            for half in range(2):
                                tt = 2 * p + half
                                nc.tensor.matmul(
                                    pr[:, half],
                                    slab[:, ko, tt * 128:(tt + 1) * 128],
                                    wv_sb[:, ko], start=(ko == 0),
                                    stop=(ko == KO - 1),
                                )

                    def drain(pr):
                        drain_pair_dve(pr, vp_all[:, s * 4 + 2 * p],
                                   vp_all[:, s * 4 + 2 * p + 1])

                    add_chain(alloc, mms, drain)

                def qk_chain(half_qk, p):
                    dst = (qt_res if half_qk == 0 else kt_res)

                    def alloc():
                        return ps_big.tile([128, 2, SLAB], F32, tag="big",
                                           name="ps_qk")

                    def mms(pr, kos):
                        for ko in kos:
                            for q in range(2):
                                co = HPC * half_qk + 2 * p + q
                                nc.tensor.matmul(
                                    pr[:, q],
                                    wqk_sb[:, ko, co * 128:(co + 1) * 128],
                                    slab[:, ko], start=(ko == 0),
                                    stop=(ko == KO - 1),
                                )

                    def drain(pr):
                        drain_pair_dve(
                            pr,
                            dst[:, 2 * p, s * SLAB:(s + 1) * SLAB],
                            dst[:, 2 * p + 1, s * SLAB:(s + 1) * SLAB])

                    add_chain(alloc, mms, drain)

                for p in range(2):
                    v_chain(p)
                for half_qk in range(2):
                    for p in range(2):
                        qk_chain(half_qk, p)
                return halves

            # ============ attention + out projection (block b) ============
            def final_proj_il(bb, aot_bb, il, fine_last=False):
                oc = oc_pool.tile([128, 4, BW], BF16)
                row = slice((4 * bb + il) * 128, (4 * bb + il + 1) * 128)
                for cb in range(4):
                    ps_f = ps_sm_pool.tile([128, BW], F32, tag="sm", name="ps_f")
                    for h in range(HPC):
                        nc.tensor.matmul(
                            ps_f[:], aot_bb[:, h, il * 128:(il + 1) * 128],
                            wout_sb[:, h, cb * BW:(cb + 1) * BW],
                            start=(h == 0), stop=(h == HPC - 1),
                        )
                    if fine_last:
                        # final row: drain halves on ACT+DVE in parallel and
                        # DMA 128-col pieces on both queues to shrink the
                        # post-compute tail (a 128-col piece is ~1.3us on one
                        # stream vs ~2.6us for 256 cols)
                        nc.scalar.activation(oc[:, cb, 0:256], ps_f[:, 0:256],
                                             mybir.ActivationFunctionType.Copy)
                        nc.vector.tensor_copy(oc[:, cb, 256:512],
                                              ps_f[:, 256:512])
                        for q in range(4):
                            eng = nc.sync if q % 2 == 0 else nc.scalar
                            eng.dma_start(
                                out[row, cb * BW + q * 128:cb * BW + q * 128 + 128],
                                oc[:, cb, q * 128:(q + 1) * 128])
                    # drains alternate ACT / DVE
                    elif cb % 2 == 0:
                        nc.scalar.activation(oc[:, cb], ps_f[:],
                                             mybir.ActivationFunctionType.Copy)
                    else:
                        nc.vector.tensor_copy(oc[:, cb], ps_f[:])
                if not fine_last:
                    # one whole-row DMA per il (sync dispatch ~600ns each)
                    nc.sync.dma_start(out[row, :], oc[:])

            aots = []

            def emit_D(b, filler=()):
                aot = aot_pool.tile([128, HPC, BW], BF16)
                aots.append(aot)
                nj = 4 * b + 4
                filler = list(filler)

                def fill():
                    # ACT-free PE filler (projection half-chains).  The fill
                    # points are spaced so consecutive halves of one chain
                    # have at most ONE other ps_big allocation between them
                    # (pool bufs=2 -- more would deadlock the PE FIFO).
                    if filler:
                        filler.pop(0)()

                pending = [None]  # deferred (den, ps_o, h) finalization

                def finish_pending():
                    if pending[0] is None:
                        return
                    den_p, ps_o_p, h_p = pending[0]
                    pending[0] = None
                    ps_n = ps_sm_pool.tile([128, BW], F32, tag="sm", name="ps_n")
                    nc.tensor.matmul(ps_n[:], ones_mat[:], den_p[:],
                                     start=True, stop=True)
                    rec = rec_pool.tile([128, BW], F32, tag="rec", name="rec")
                    nc.vector.reciprocal_approx_fast(rec[:], ps_n[:])
                    nc.vector.tensor_mul(aot[:, h_p], ps_o_p[:], rec[:])

                for h in range(HPC):
                    qt_b = qt_res[:, h, b * BW:(b + 1) * BW]
                    ps_o = ps_o_pool.tile([128, BW], F32, tag="ps_o")
                    den = den_pool.tile([128, BW], BF16, name="den")
                    ats = []          # (at AP, c0) per j
                    av_done = [0]

                    def emit_avs(upto):
                        # av matmuls lag the exp stream; ps_o accumulates
                        while av_done[0] <= upto:
                            j = av_done[0]
                            pat, pc0 = ats[j]
                            nc.tensor.matmul(
                                ps_o[:, pc0:],
                                vp_all[:, j, h * DH:(h + 1) * DH],
                                pat[:, pc0:], start=(j == 0),
                                stop=(j == nj - 1))
                            av_done[0] += 1

                    def emit_den(j):
                        pat, pc0 = ats[j]
                        if j == 0:
                            nc.vector.tensor_copy(den[:, pc0:], pat[:, pc0:])
                        else:
                            nc.vector.tensor_add(den[:, pc0:], den[:, pc0:],
                                                 pat[:, pc0:])

                    # non-diagonal tiles in pairs: 2 score matmuls into the
                    # halves of one 2-bank pair, ONE 1024-wide exp
                    for jp in range(0, 4 * b, 2):
                        ps_s = ps_big.tile([128, 2, BW], F32, tag="big",
                                           name="ps_s")
                        for half in range(2):
                            j = jp + half
                            nc.tensor.matmul(
                                ps_s[:, half],
                                kt_res[:, h, j * 128:(j + 1) * 128],
                                qt_b[:], start=True, stop=True)
                        at = at_pool.tile([128, 2, BW], BF16)
                        nc.scalar.activation(
                            at[:], ps_s[:],
                            mybir.ActivationFunctionType.Exp,
                            bias=bias_sb[:], scale=SCALE,
                        )
                        ats.append((at[:, 0], 0))
                        ats.append((at[:, 1], 0))
                        emit_den(jp)
                        emit_den(jp + 1)
                        if jp == 2:
                            # the deferred (h-1) ones-matmul/rec/rescale: by
                            # now the DVE den chain of h-1 has long drained
                            finish_pending()
                        emit_avs(jp - 1)
                        fill()
                    # diagonal tiles (g=0..3): column-trimmed singles
                    for g in range(4):
                        j = 4 * b + g
                        c0 = 128 * g
                        half = g % 2
                        if half == 0:
                            ps_s = ps_big.tile([128, 2, BW], F32, tag="big",
                                               name="ps_sd")
                            at = at_pool.tile([128, 2, BW], BF16)
                        nc.tensor.matmul(
                            ps_s[:, half, c0:],
                            kt_res[:, h, j * 128:(j + 1) * 128],
                            qt_b[:, c0:], start=True, stop=True)
                        nc.scalar.activation(
                            at[:, half, c0:], ps_s[:, half, c0:],
                            mybir.ActivationFunctionType.Exp,
                            bias=bias_sb[:], scale=SCALE,
                        )
                        # causal: zero attnT where tk > tq (gpsimd idle)
                        nc.gpsimd.affine_select(
                            out=at[:, half, c0:], in_=at[:, half, c0:],
                            pattern=[[1, BW - c0]],
                            compare_op=mybir.AluOpType.is_ge, fill=0.0,
                            base=0,
                            channel_multiplier=-1,
                        )
                        ats.append((at[:, half], c0))
                        emit_den(j)
                        if b == 0 and g == 2:
                            # b=0 has no pair loop; finish (h-1) here instead
                            finish_pending()
                        emit_avs(j - 1)
                        if half == 1:
                            fill()
                    emit_avs(nj - 1)
                    fill()
                    finish_pending()  # no-op unless a finish point was missed
                    pending[0] = (den, ps_o, h)
                    # interleave block b-1's out projection one il per head so
                    # its psum drains spread across the block's exp stream
                    if b > 0:
                        final_proj_il(b - 1, aots[b - 1], h)
                finish_pending()
                for run in filler:
                    run()

            slab0 = emit_A_dma(0)
            emit_A_compute(0, slab0)
            slab1 = emit_A_dma(1)
            emit_A_compute(1, slab1)
            slab2 = emit_A_dma(2)
            emit_D(0, make_half_chains(2, slab2))
            slab3 = emit_A_dma(3)
            emit_D(1, make_half_chains(3, slab3))
            emit_D(2)
            emit_D(3)
            for il in range(4):
                final_proj_il(NB - 1, aots[NB - 1], il, fine_last=(il == 3))

    nc.compile()
    return nc


_NC = None


def _get_nc():
    global _NC
    if _NC is None:
        _NC = build_nc()
    return _NC


def kernel(x, mask, Wqkv, Wout, _trace=False):
    assert x.shape == (B, T, C) and Wqkv.shape == (C, 3 * C) and Wout.shape == (C, C)
    import ml_dtypes
    bf16 = ml_dtypes.bfloat16
    nc = _get_nc()

    def pack_rows(a):
        # [KO*128, M] -> [128, KO*M]: row ko*128+p lands at [p, ko*M:(ko+1)*M]
        ko = a.shape[0] // 128
        return np.ascontiguousarray(
            a.reshape(ko, 128, -1).transpose(1, 0, 2).reshape(128, -1))

    # xt[p, s, ko, tl] = x[b][s*SLAB+tl, ko*128+p]
    xt = [np.ascontiguousarray(
              x[b].T.reshape(KO, 128, NSLAB, SLAB).transpose(1, 2, 0, 3)
              .reshape(128, -1)).astype(bf16) for b in range(B)]
    in_maps = []
    for c in range(8):
        b, g = c // 4, c % 4
        h0 = g * HPC * DH          # column offset of this core's heads
        wqk_c = pack_rows(np.concatenate(
            [Wqkv[:, h0:h0 + HPC * DH],
             Wqkv[:, C + h0:C + h0 + HPC * DH]], axis=1)).astype(bf16)
        wv_c = pack_rows(Wqkv[:, 2 * C + h0:2 * C + h0 + HPC * DH]).astype(bf16)
        wout_c = pack_rows(Wout[h0:h0 + HPC * DH, :]).astype(bf16)
        in_maps.append({"xt": xt[b], "wqk": wqk_c, "wv": wv_c, "wout": wout_c})

    kwargs = {}
    if _trace:
        import os
        kwargs = dict(trace=True, tmpdir=os.environ.get("KERNEL_TRACE_DIR"))
    res = run_bass_kernel_spmd(nc, in_maps, core_ids=list(range(8)), **kwargs)

    outs = np.zeros((B, T, C), dtype=np.float64)
    for c in range(8):
        outs[c // 4] += res.results[c]["out"].astype(np.float64)
    result = outs.astype(np.float32)
    if _trace:
        return result, res
    return result
